# revision 9
# baseline (speedup 1.0000x reference)
"""Trainium2 Bass kernel for nn_Decoder (show-attend-tell greedy decoder).

Sharding across 8 NeuronCores:
  - attention path batch-sharded (8 rows/core, enc slice resident in SBUF)
  - GRU state path replicated (full batch 64; matmul cost is N-bound so
    replication costs nothing and avoids an extra collective)
  - vocab projection sharded column-wise (fc_w.T slice [512,4000] resident in
    SBUF per core); exact fp32 greedy argmax via AllGather of per-core top-1.
Per step: AllGather(context [8,512] -> [64,512]) and AllGather(argmax
candidates [64,2] -> [8,64,2]).
"""
import numpy as np

import concourse.bass as bass
import concourse.mybir as mybir
import concourse.tile as tile
from concourse import bacc
from concourse.bass_utils import run_bass_kernel_spmd
from concourse.masks import make_identity

F32 = mybir.dt.float32
U32 = mybir.dt.uint32
U8 = mybir.dt.uint8
AF = mybir.ActivationFunctionType
OP = mybir.AluOpType
AX = mybir.AxisListType

NC = 8           # cores
B = 64           # batch
BL = B // NC     # batch rows per core
P = 400          # pixels
ED = 512         # encoder dim
DD = 512         # decoder dim
EMB = 256        # embedding dim
AD = 256         # attention dim
CD = 256         # conv dim
V = 32000        # vocab
VL = V // NC     # vocab slice per core
NT = 8           # fc n-tiles
VT = VL // NT    # 500
BIG = 1.0e9


def build(Tmax: int):
    nc = bacc.Bacc(num_devices=NC)

    def din(name, shape, dt=F32):
        return nc.dram_tensor(name, shape, dt, kind="ExternalInput")

    encP_d = din("encP", [128, 4, BL, ED])        # [ki, pc, b, d]; p=pc*128+ki, zero-padded
    fcwT_d = din("fcwT", [128, 4, VL])            # fc_w slice .T  [dd, v]
    wih1_d = din("wih1", [128, 2, 3 * DD])        # gru1 w_ih.T  [emb, 3dd]
    whh1_d = din("whh1", [128, 4, 3 * DD])
    wih2_d = din("wih2", [128, 4, 3 * DD])
    whh2_d = din("whh2", [128, 4, 3 * DD])
    datwT_d = din("datwT", [128, 4, AD])          # dec_att_w.T [dd, a]
    cvwT_d = din("cvwT", [128, 4, CD])            # conv_w.T [p(pad 512), cd]
    cawT_d = din("cawT", [128, 2, AD])            # conv_att_w.T [cd, a]
    eawT_d = din("eawT", [128, 4, AD])            # enc_att_w.T [ed, a]
    wsel_d = din("wsel", [128, 2, BL, BL])        # full_att_w selection columns
    s0_2d_d = din("s0_2d", [B, DD])
    s0T_d = din("s0T", [128, 4, B])
    emb0T_d = din("emb0T", [128, 2, B])
    eabT_d = din("eabT", [128, 2])
    ab0T_d = din("ab0T", [128, 2])
    ab1T_d = din("ab1T", [128, 2])
    Sown_d = din("Sown", [B, BL])
    vofff_d = din("vofff", [B, 1])
    maskpT_d = din("maskpT", [B, Tmax])
    maskaT_d = din("maskaT", [BL, Tmax])
    embed_d = din("embed", [V, EMB])

    preds_o = nc.dram_tensor("preds", [B, Tmax, VL], F32, kind="ExternalOutput")
    alphas_o = nc.dram_tensor("alphas", [BL, Tmax, P], F32, kind="ExternalOutput")

    with tile.TileContext(nc) as tc:
        with (
            tc.tile_pool(name="cst", bufs=1) as cst,
            tc.tile_pool(name="strm", bufs=2) as strm,
            tc.tile_pool(name="att1s", bufs=2) as att1p,
            tc.tile_pool(name="wk", bufs=1) as wk,
            tc.tile_pool(name="ps", bufs=1, space="PSUM") as ps,
            tc.tile_pool(name="dram", bufs=1, space="DRAM") as dram,
        ):
            # ---------- resident tensors ----------
            encP = cst.tile([128, 4, BL, ED], F32)
            nc.sync.dma_start(encP[:], encP_d[:])
            fcwT = cst.tile([128, 4, VL], F32)
            nc.sync.dma_start(fcwT[:], fcwT_d[:])
            datwT = cst.tile([128, 4, AD], F32)
            nc.sync.dma_start(datwT[:], datwT_d[:])
            cvwT = cst.tile([128, 4, CD], F32)
            nc.sync.dma_start(cvwT[:], cvwT_d[:])
            cawT = cst.tile([128, 2, AD], F32)
            nc.sync.dma_start(cawT[:], cawT_d[:])
            wsel = cst.tile([128, 2, BL, BL], F32)
            nc.sync.dma_start(wsel[:], wsel_d[:])
            eabT = cst.tile([128, 2], F32)
            nc.sync.dma_start(eabT[:], eabT_d[:])
            ab0T = cst.tile([128, 2], F32)
            nc.sync.dma_start(ab0T[:], ab0T_d[:])
            ab1T = cst.tile([128, 2], F32)
            nc.sync.dma_start(ab1T[:], ab1T_d[:])
            Sown = cst.tile([B, BL], F32)
            nc.sync.dma_start(Sown[:], Sown_d[:])
            vofff = cst.tile([B, 1], F32)
            nc.sync.dma_start(vofff[:], vofff_d[:])
            maskpT = cst.tile([B, Tmax], F32)
            nc.sync.dma_start(maskpT[:], maskpT_d[:])
            maskaT = cst.tile([BL, Tmax], F32)
            nc.sync.dma_start(maskaT[:], maskaT_d[:])
            ident = cst.tile([128, 128], F32)
            make_identity(nc, ident[:])
            bigt = cst.tile([B, NT * 8], F32)
            nc.vector.memset(bigt[:], BIG)

            st2d = [cst.tile([B, DD], F32, name=f"st2d{i}") for i in range(2)]
            stT = [cst.tile([128, 4, B], F32, name=f"stT{i}") for i in range(2)]
            embT = [cst.tile([128, 2, B], F32, name=f"embT{i}") for i in range(2)]
            cvT = cst.tile([128, 4, BL], F32)
            alphaT_sb = cst.tile([128, 4, BL], F32)
            alphasel = cst.tile([128, 4, BL, BL], F32)
            nc.vector.memset(cvT[:], 0.0)
            nc.vector.memset(alphaT_sb[:], 0.0)
            nc.vector.memset(alphasel[:], 0.0)
            nc.sync.dma_start(st2d[0][:], s0_2d_d[:])
            nc.sync.dma_start(stT[0][:], s0T_d[:])
            nc.sync.dma_start(embT[0][:], emb0T_d[:])

            att1_h = dram.tile([BL, 2, 128, P], F32)
            agc_in = dram.tile([BL, DD], F32)
            agc_out = dram.tile([B, DD], F32)
            agk_in = dram.tile([B, 2], F32)
            agk_out = dram.tile([NC * B, 2], F32)
            groups = [list(range(NC))]

            # psum tags: fc(2x2K) gg2(2x2K) t64(1x2K) tsm(1x2K) tny(2x2K) = 16K
            def ps_fc():
                return ps.tile([B, VT], F32, tag="fc", bufs=2, name="psfc")

            def ps_gg(shape, name):
                return ps.tile(shape, F32, tag="gg2", bufs=2, name=name)

            def ps_t64(shape, name):
                return ps.tile(shape, F32, tag="t64", bufs=1, name=name)

            def ps_tsm(shape, name):
                return ps.tile(shape, F32, tag="tsm", bufs=1, name=name)

            def ps_tny(shape, name):
                return ps.tile(shape, F32, tag="tny", bufs=2, name=name)

            # ---------- init: att1 = enc @ enc_att_w.T + enc_att_b ----------
            eaw = strm.tile([128, 4, AD], F32, tag="wst", name="eaw")
            nc.sync.dma_start(eaw[:], eawT_d[:])
            for b in range(BL):
                a1p = [ps_gg([128, ED], f"a1p{ac}") for ac in range(2)]
                for dc in range(4):
                    tp = ps_tsm([128, ED], "initT")
                    for pc in range(4):
                        nc.tensor.transpose(
                            tp[:, pc * 128:(pc + 1) * 128],
                            encP[:, pc, b, dc * 128:(dc + 1) * 128],
                            ident[:])
                    et = wk.tile([128, ED], F32, tag="encT", bufs=2, name="et")
                    nc.scalar.copy(et[:], tp[:])
                    for ac in range(2):
                        nc.tensor.matmul(
                            a1p[ac][:],
                            eaw[:, dc, ac * 128:(ac + 1) * 128],
                            et[:], start=(dc == 0), stop=(dc == 3))
                for ac in range(2):
                    a1s = wk.tile([128, ED], F32, tag="a1s", bufs=1, name="a1s")
                    nc.scalar.activation(a1s[:], a1p[ac][:], AF.Identity,
                                         bias=eabT[:, ac:ac + 1], scale=1.0)
                    nc.sync.dma_start(att1_h[b, ac], a1s[:, :P])

            # ---------- decode loop ----------
            for t in range(Tmax):
                cur, nxt = t % 2, (t + 1) % 2
                sT = stT[cur]
                s2 = st2d[cur]
                eT = embT[cur]
                last = (t == Tmax - 1)

                # ===== B. attention on own rows, uses s_{t-1} =====
                sop = ps_tsm([BL, DD], "sop")
                nc.tensor.matmul(sop[:], Sown[:], s2[:], start=True, stop=True)
                so = wk.tile([BL, DD], F32, tag="so", name="so")
                nc.scalar.copy(so[:], sop[:])
                soTp = ps_tny([128, 4, BL], "soTp")
                for kc in range(4):
                    nc.tensor.transpose(soTp[:, kc, :],
                                        so[:, kc * 128:(kc + 1) * 128],
                                        ident[:BL, :BL])
                soT = wk.tile([128, 4, BL], F32, tag="soT", name="soT")
                nc.vector.tensor_copy(soT[:], soTp[:])
                a2p = ps_tsm([BL, AD], "a2p")
                for kc in range(4):
                    nc.tensor.matmul(a2p[:], soT[:, kc, :], datwT[:, kc, :],
                                     start=(kc == 0), stop=(kc == 3))
                a2 = wk.tile([BL, AD], F32, tag="a2", name="a2")
                nc.scalar.copy(a2[:], a2p[:])
                a23p = ps_tny([128, 2, BL], "a23p")
                for ac in range(2):
                    nc.tensor.transpose(a23p[:, ac, :],
                                        a2[:, ac * 128:(ac + 1) * 128],
                                        ident[:BL, :BL])
                if t >= 1:
                    cp = ps_tsm([BL, CD], "cp")
                    for pc in range(4):
                        nc.tensor.matmul(cp[:], cvT[:, pc, :], cvwT[:, pc, :],
                                         start=(pc == 0), stop=(pc == 3))
                    csb = wk.tile([BL, CD], F32, tag="csb", name="csb")
                    nc.scalar.copy(csb[:], cp[:])
                    cTp = ps_tny([128, 2, BL], "cTp")
                    for kc in range(2):
                        nc.tensor.transpose(cTp[:, kc, :],
                                            csb[:, kc * 128:(kc + 1) * 128],
                                            ident[:BL, :BL])
                    cT = wk.tile([128, 2, BL], F32, tag="cT", name="cT")
                    nc.vector.tensor_copy(cT[:], cTp[:])
                    for ac in range(2):
                        for kc in range(2):
                            nc.tensor.matmul(
                                a23p[:, ac, :],
                                cawT[:, kc, ac * 128:(ac + 1) * 128],
                                cT[:, kc, :], start=False, stop=(kc == 1))
                a23 = wk.tile([128, 2, BL], F32, tag="a23", name="a23")
                abT = ab1T if t >= 1 else ab0T
                for ac in range(2):
                    nc.scalar.activation(a23[:, ac, :], a23p[:, ac, :],
                                         AF.Identity, bias=abT[:, ac:ac + 1],
                                         scale=1.0)
                scp = ps_tsm([BL, P], "scp")
                for b in range(BL):
                    a1t = att1p.tile([128, 2, P], F32, tag="a1t", name="a1t")
                    nc.sync.dma_start(
                        a1t[:], att1_h[b].rearrange("a k p -> k a p"))
                    for ac in range(2):
                        et2 = wk.tile([128, P], F32, tag="et2", bufs=1, name="et2")
                        nc.scalar.activation(et2[:], a1t[:, ac, :], AF.Tanh,
                                             bias=a23[:, ac, b:b + 1], scale=1.0)
                        nc.tensor.matmul(scp[:], wsel[:, ac, b, :], et2[:],
                                         start=(b == 0 and ac == 0),
                                         stop=(b == BL - 1 and ac == 1))
                nmax = wk.tile([BL, 1], F32, tag="nmax", name="nmax")
                nc.vector.tensor_reduce(nmax[:], scp[:], axis=AX.X, op=OP.max)
                nc.vector.tensor_scalar_mul(nmax[:], nmax[:], -1.0)
                exps = wk.tile([BL, P], F32, tag="exps", name="exps")
                sume = wk.tile([BL, 1], F32, tag="sume", name="sume")
                nc.scalar.activation(exps[:], scp[:], AF.Exp, bias=nmax[:],
                                     scale=1.0, accum_out=sume[:])
                rsum = wk.tile([BL, 1], F32, tag="rsum", name="rsum")
                nc.vector.reciprocal(rsum[:], sume[:])
                nc.vector.tensor_scalar_mul(exps[:], exps[:], rsum[:])  # alpha
                aout = wk.tile([BL, P], F32, tag="aout", name="aout")
                nc.scalar.activation(aout[:], exps[:], AF.Copy,
                                     scale=maskaT[:, t:t + 1])
                nc.sync.dma_start(alphas_o[:, t, :], aout[:])
                atp = ps_tny([128, 4, BL], "atp")
                for pc in range(3):
                    nc.tensor.transpose(atp[:, pc, :],
                                        exps[:, pc * 128:(pc + 1) * 128],
                                        ident[:BL, :BL])
                nc.tensor.transpose(atp[:16, 3, :], exps[:, 384:400],
                                    ident[:BL, :BL])
                for pc in range(3):
                    nc.vector.tensor_copy(alphaT_sb[:, pc, :], atp[:, pc, :])
                    nc.vector.tensor_copy(
                        alphasel[:, pc].rearrange("k a b -> k (a b)")[:, 0:64:9],
                        atp[:, pc, :])
                nc.vector.tensor_copy(alphaT_sb[:16, 3, :], atp[:16, 3, :])
                nc.vector.tensor_copy(
                    alphasel[:16, 3].rearrange("k a b -> k (a b)")[:, 0:64:9],
                    atp[:16, 3, :])
                ctxp = ps_tsm([BL, ED], "ctxp")
                for b in range(BL):
                    for pc in range(4):
                        nc.tensor.matmul(
                            ctxp[:], alphasel[:, pc, b, :],
                            encP[:, pc, b, :],
                            start=(b == 0 and pc == 0),
                            stop=(b == BL - 1 and pc == 3))
                ctx = wk.tile([BL, ED], F32, tag="ctx", name="ctx")
                nc.scalar.copy(ctx[:], ctxp[:])
                nc.vector.tensor_add(cvT[:], cvT[:], alphaT_sb[:])
                nc.sync.dma_start(agc_in[:], ctx[:])
                nc.gpsimd.collective_compute(
                    "AllGather", OP.bypass, replica_groups=groups,
                    ins=[agc_in[:].opt()], outs=[agc_out[:].opt()])
                ctx2 = wk.tile([B, ED], F32, tag="ctx2", name="ctx2")
                nc.sync.dma_start(ctx2[:], agc_out[:])
                cxTp = ps_t64([128, 4, B], "cxTp")
                for kc in range(4):
                    nc.tensor.transpose(cxTp[:, kc, :],
                                        ctx2[:, kc * 128:(kc + 1) * 128],
                                        ident[:B, :B])
                ctxT = wk.tile([128, 4, B], F32, tag="ctxT", name="ctxT")
                nc.vector.tensor_copy(ctxT[:], cxTp[:])

                # ===== C/D. the two GRU cells (full batch) =====
                def gru(xT, kx, wih_d_, hT, h2, whh_d_, name):
                    def wload(wd, kn, g, sfx):
                        wt = strm.tile([128, kn, DD], F32, tag="wst",
                                       name=f"w{name}{sfx}")
                        nc.sync.dma_start(wt[:],
                                          wd[:, :, g * DD:(g + 1) * DD])
                        return wt

                    def gate_sum(g, gname):
                        # gi + gh accumulated in one psum tile by the matmuls
                        wi = wload(wih_d_, kx, g, f"i{g}")
                        wh = wload(whh_d_, 4, g, f"h{g}")
                        gp = ps_gg([B, DD], gname)
                        for k in range(kx):
                            nc.tensor.matmul(gp[:], xT[:, k, :], wi[:, k, :],
                                             start=(k == 0), stop=False)
                        for k in range(4):
                            nc.tensor.matmul(gp[:], hT[:, k, :], wh[:, k, :],
                                             start=False, stop=(k == 3))
                        return gp

                    # r gate
                    gp = gate_sum(0, f"gr{name}")
                    r = wk.tile([B, DD], F32, tag="rga", name="rga")
                    nc.scalar.activation(r[:], gp[:], AF.Sigmoid)
                    # n gate (separate i/h parts; keep r alive)
                    wi = wload(wih_d_, kx, 2, "i2")
                    gpi = ps_gg([B, DD], f"gni{name}")
                    for k in range(kx):
                        nc.tensor.matmul(gpi[:], xT[:, k, :], wi[:, k, :],
                                         start=(k == 0), stop=(k == kx - 1))
                    wh = wload(whh_d_, 4, 2, "h2")
                    gph = ps_gg([B, DD], f"gnh{name}")
                    for k in range(4):
                        nc.tensor.matmul(gph[:], hT[:, k, :], wh[:, k, :],
                                         start=(k == 0), stop=(k == 3))
                    n = wk.tile([B, DD], F32, tag="nga", name="nga")
                    nc.vector.tensor_tensor(n[:], r[:], gph[:], op=OP.mult)
                    nc.vector.tensor_add(n[:], n[:], gpi[:])
                    nc.scalar.activation(n[:], n[:], AF.Tanh)
                    # z gate
                    gp = gate_sum(1, f"gz{name}")
                    z = wk.tile([B, DD], F32, tag="zga", name="zga")
                    nc.scalar.activation(z[:], gp[:], AF.Sigmoid)
                    # out = (h - n) * z + n
                    nc.vector.tensor_tensor(r[:], h2[:], n[:], op=OP.subtract)
                    nc.vector.tensor_tensor(r[:], r[:], z[:], op=OP.mult)
                    out2 = wk.tile([B, DD], F32, tag=f"o2{name}", name=f"o2{name}")
                    nc.vector.tensor_add(out2[:], r[:], n[:])
                    oTp = ps_t64([128, 4, B], f"oT{name}")
                    for kc in range(4):
                        nc.tensor.transpose(oTp[:, kc, :],
                                            out2[:, kc * 128:(kc + 1) * 128],
                                            ident[:B, :B])
                    return out2, oTp

                sh2, shTp = gru(eT, 2, wih1_d, sT, s2, whh1_d, "1")
                shT = wk.tile([128, 4, B], F32, tag="shT", name="shT")
                nc.vector.tensor_copy(shT[:], shTp[:])
                new2, newTp = gru(ctxT, 4, wih2_d, shT, sh2, whh2_d, "2")
                nc.vector.tensor_copy(st2d[nxt][:], new2[:])
                nc.scalar.copy(stT[nxt][:], newTp[:])

                # ===== A. vocab projection + greedy argmax feedback =====
                svals = wk.tile([B, NT * 8], F32, tag="svals", name="svals")
                sidxf = wk.tile([B, NT * 8], F32, tag="sidxf", name="sidxf")
                for nt in range(NT):
                    pt = ps_fc()
                    for k in range(4):
                        nc.tensor.matmul(
                            pt[:], stT[nxt][:, k, :], fcwT[:, k, nt * VT:(nt + 1) * VT],
                            start=(k == 0), stop=(k == 3))
                    pr = wk.tile([B, VT], F32, tag="pr", bufs=2, name="pr")
                    if nt % 2 == 0:
                        nc.scalar.activation(pr[:], pt[:], AF.Copy,
                                             scale=maskpT[:, t:t + 1])
                    else:
                        nc.vector.tensor_scalar_mul(pr[:], pt[:],
                                                    maskpT[:, t:t + 1])
                    nc.sync.dma_start(preds_o[:, t, nt * VT:(nt + 1) * VT], pr[:])
                    if not last:
                        sl = slice(nt * 8, (nt + 1) * 8)
                        nc.vector.max(svals[:, sl], pr[:])
                        mi8 = wk.tile([B, 8], U32, tag="mi8", name="mi8")
                        nc.vector.max_index(mi8[:], svals[:, sl], pr[:])
                        nc.vector.tensor_copy(sidxf[:, sl], mi8[:])   # u32->f32
                        nc.vector.tensor_scalar_add(sidxf[:, sl], sidxf[:, sl],
                                                    float(nt * VT))
                if not last:
                    # within-core top-1 (value max, tie -> smallest index)
                    gmax = wk.tile([B, 1], F32, tag="gmax", name="gmax")
                    nc.vector.tensor_reduce(gmax[:], svals[:], axis=AX.X, op=OP.max)
                    meq = wk.tile([B, NT * 8], U8, tag="meq", name="meq")
                    nc.vector.tensor_scalar(meq[:], svals[:], gmax[:], None,
                                            op0=OP.is_equal)
                    seli = wk.tile([B, NT * 8], F32, tag="seli", name="seli")
                    nc.vector.select(seli[:], meq[:], sidxf[:], bigt[:])
                    lidx = wk.tile([B, 1], F32, tag="lidx", name="lidx")
                    nc.vector.tensor_reduce(lidx[:], seli[:], axis=AX.X, op=OP.min)
                    cand = wk.tile([B, 2], F32, tag="cand", name="cand")
                    nc.vector.tensor_copy(cand[:, 0:1], gmax[:])
                    nc.vector.tensor_tensor(cand[:, 1:2], lidx[:], vofff[:],
                                            op=OP.add)
                    nc.sync.dma_start(agk_in[:], cand[:])
                    nc.gpsimd.collective_compute(
                        "AllGather", OP.bypass, replica_groups=groups,
                        ins=[agk_in[:].opt()], outs=[agk_out[:].opt()])
                    call = wk.tile([B, NC, 2], F32, tag="call", name="call")
                    nc.sync.dma_start(
                        call[:], agk_out[:].rearrange("(c b) k -> b c k", c=NC))
                    g2 = wk.tile([B, 1], F32, tag="g2", name="g2")
                    nc.vector.tensor_reduce(g2[:], call[:, :, 0], axis=AX.X,
                                            op=OP.max)
                    m2 = wk.tile([B, NC], U8, tag="m2", name="m2")
                    nc.vector.tensor_scalar(m2[:], call[:, :, 0], g2[:], None,
                                            op0=OP.is_equal)
                    s2i = wk.tile([B, NC], F32, tag="s2i", name="s2i")
                    nc.vector.select(s2i[:], m2[:], call[:, :, 1], bigt[:, :NC])
                    wf = wk.tile([B, 1], F32, tag="wf", name="wf")
                    nc.vector.tensor_reduce(wf[:], s2i[:], axis=AX.X, op=OP.min)
                    wu = wk.tile([B, 1], U32, tag="wu", name="wu")
                    nc.vector.tensor_copy(wu[:], wf[:])
                    embn = wk.tile([B, EMB], F32, tag="embn", name="embn")
                    nc.gpsimd.indirect_dma_start(
                        out=embn[:], out_offset=None, in_=embed_d[:],
                        in_offset=bass.IndirectOffsetOnAxis(ap=wu[:, :1], axis=0))
                    etp = ps_t64([128, 2, B], "etp")
                    for kc in range(2):
                        nc.tensor.transpose(
                            etp[:, kc, :], embn[:, kc * 128:(kc + 1) * 128],
                            ident[:B, :B])
                    nc.scalar.copy(embT[nxt][:], etp[:])


    nc.finalize()
    return nc


_CACHE = {}


def _get_nc(Tmax):
    if Tmax not in _CACHE:
        _CACHE[Tmax] = build(Tmax)
    return _CACHE[Tmax]


def _chunkT(w, kchunks):
    """[out, in] weight -> lhsT/rhs SBUF layout [128, kchunks, out]."""
    o, i = w.shape
    assert i == kchunks * 128
    return np.ascontiguousarray(
        w.T.reshape(kchunks, 128, o).transpose(1, 0, 2)).astype(np.float32)


def kernel(encoder_out, encoded_captions, caption_lengths, embedding,
           gru1_w_ih, gru1_w_hh, gru1_b_ih, gru1_b_hh,
           gru2_w_ih, gru2_w_hh, gru2_b_ih, gru2_b_hh,
           enc_att_w, enc_att_b, dec_att_w, dec_att_b,
           conv_w, conv_b, conv_att_w, conv_att_b,
           full_att_w, full_att_b, s_w, s_b, fc_w, fc_b, _trace=False):
    encoder_out = np.asarray(encoder_out, np.float32)
    caption_lengths = np.asarray(caption_lengths)
    encoded_captions = np.asarray(encoded_captions)
    embedding = np.asarray(embedding, np.float32)
    f32 = lambda x: np.asarray(x, np.float32)
    gru1_w_ih, gru1_w_hh, gru2_w_ih, gru2_w_hh = map(
        f32, (gru1_w_ih, gru1_w_hh, gru2_w_ih, gru2_w_hh))
    enc_att_w, dec_att_w, conv_w, conv_att_w, full_att_w, s_w, fc_w = map(
        f32, (enc_att_w, dec_att_w, conv_w, conv_att_w, full_att_w, s_w, fc_w))
    for nm, bb in (("gru biases", (gru1_b_ih, gru1_b_hh, gru2_b_ih, gru2_b_hh)),
                   ("fc_b", (fc_b,))):
        for x in bb:
            assert not np.any(np.asarray(x)), f"nonzero {nm} not supported"

    sort_ind = np.argsort(-caption_lengths, kind="stable")
    lens = caption_lengths[sort_ind]
    enc = encoder_out[sort_ind]
    caps = encoded_captions[sort_ind]
    Tmax = int(lens.max())

    mean = enc.mean(axis=1, dtype=np.float32).astype(np.float32)
    s0 = (mean @ s_w.T + np.asarray(s_b, np.float32)).astype(np.float32)
    s0T = _chunkT(s0, 4)
    emb0 = np.ascontiguousarray(embedding[np.asarray(caps[:, 0], np.int64)])
    emb0T = _chunkT(emb0, 2)

    wih1 = _chunkT(gru1_w_ih, 2)
    whh1 = _chunkT(gru1_w_hh, 4)
    wih2 = _chunkT(gru2_w_ih, 4)
    whh2 = _chunkT(gru2_w_hh, 4)
    datwT = _chunkT(dec_att_w, 4)
    cawT = _chunkT(conv_att_w, 2)
    eawT = _chunkT(enc_att_w, 4)
    conv_w_pad = np.zeros((CD, 512), np.float32)
    conv_w_pad[:, :P] = conv_w
    cvwT = _chunkT(conv_w_pad, 4)
    eabT = np.ascontiguousarray(np.asarray(enc_att_b, np.float32).reshape(2, 128).T)
    ab0 = np.asarray(dec_att_b, np.float32)
    ab1 = (ab0 + np.asarray(conv_att_b, np.float32)
           + conv_att_w @ np.asarray(conv_b, np.float32)).astype(np.float32)
    ab0T = np.ascontiguousarray(ab0.reshape(2, 128).T)
    ab1T = np.ascontiguousarray(ab1.reshape(2, 128).T)
    wv = full_att_w[0].astype(np.float32)
    wsel = np.zeros((128, 2, BL, BL), np.float32)
    for ac in range(2):
        for b in range(BL):
            wsel[:, ac, b, b] = wv[ac * 128:(ac + 1) * 128]
    maskpT = (np.arange(Tmax)[None, :] < np.asarray(lens)[:, None]).astype(np.float32)
    maskpT = np.ascontiguousarray(maskpT)

    nc = _get_nc(Tmax)

    in_maps = []
    for c in range(NC):
        rows = slice(c * BL, (c + 1) * BL)
        enc_own = enc[rows]
        encP = np.zeros((128, 4, BL, ED), np.float32)
        ep = enc_own.transpose(1, 0, 2)   # [p, b, d]
        for pc in range(4):
            n = min(128, P - pc * 128)
            encP[:n, pc] = ep[pc * 128:pc * 128 + n]
        fcwT = _chunkT(fc_w[c * VL:(c + 1) * VL], 4)
        Sown = np.zeros((B, BL), np.float32)
        for j in range(BL):
            Sown[c * BL + j, j] = 1.0
        in_maps.append(dict(
            encP=encP, fcwT=fcwT, wih1=wih1, whh1=whh1, wih2=wih2, whh2=whh2,
            datwT=datwT, cvwT=cvwT, cawT=cawT, eawT=eawT, wsel=wsel,
            s0_2d=s0, s0T=s0T, emb0T=emb0T, eabT=eabT, ab0T=ab0T, ab1T=ab1T,
            Sown=Sown,
            vofff=np.full((B, 1), float(c * VL), np.float32),
            maskpT=maskpT,
            maskaT=np.ascontiguousarray(maskpT[rows]),
            embed=embedding,
        ))

    res = run_bass_kernel_spmd(nc, in_maps, core_ids=list(range(NC)),
                               trace=_trace)
    predictions = np.concatenate(
        [res.results[c]["preds"] for c in range(NC)], axis=2)
    alphas = np.concatenate(
        [res.results[c]["alphas"] for c in range(NC)], axis=0)

    out = (predictions, caps, lens, alphas, sort_ind.astype(np.int32))
    if _trace:
        return out, res
    return out


# revision 10
# speedup vs baseline: 1.7317x; 1.7317x over previous
"""Trainium2 Bass kernel for nn_Decoder (show-attend-tell greedy decoder).

Sharding across 8 NeuronCores:
  - attention path batch-sharded (8 rows/core, enc slice resident in SBUF)
  - GRU state path replicated (full batch 64; matmul cost is N-bound so
    replication costs nothing and avoids an extra collective)
  - vocab projection sharded column-wise (fc_w.T slice [512,4000] resident in
    SBUF per core); exact fp32 greedy argmax via AllGather of per-core top-1.
Per step: AllGather(context [8,512] -> [64,512]) and AllGather(argmax
candidates [64,2] -> [8,64,2]).
"""
import os

import numpy as np

import concourse.bass as bass
import concourse.mybir as mybir
import concourse.tile as tile
from concourse import bacc
from concourse.bass_utils import run_bass_kernel_spmd
from concourse.masks import make_identity

F32 = mybir.dt.float32
U32 = mybir.dt.uint32
U8 = mybir.dt.uint8
AF = mybir.ActivationFunctionType
OP = mybir.AluOpType
AX = mybir.AxisListType

NC = 8           # cores
B = 64           # batch
BL = B // NC     # batch rows per core
P = 400          # pixels
ED = 512         # encoder dim
DD = 512         # decoder dim
EMB = 256        # embedding dim
AD = 256         # attention dim
CD = 256         # conv dim
V = 32000        # vocab
VL = V // NC     # vocab slice per core
NT = 8           # fc n-tiles
VT = VL // NT    # 500
BIG = 1.0e9


def build(Tmax: int):
    nc = bacc.Bacc(num_devices=NC)

    def din(name, shape, dt=F32):
        return nc.dram_tensor(name, shape, dt, kind="ExternalInput")

    encP_d = din("encP", [128, 4, BL, ED])        # [ki, pc, b, d]; p=pc*128+ki, zero-padded
    fcwT_d = din("fcwT", [128, 4, VL])            # fc_w slice .T  [dd, v]
    wih1_d = din("wih1", [128, 2, 3 * DD])        # gru1 w_ih.T  [emb, 3dd]
    whh1_d = din("whh1", [128, 4, 3 * DD])
    wih2_d = din("wih2", [128, 4, 3 * DD])
    whh2_d = din("whh2", [128, 4, 3 * DD])
    datwT_d = din("datwT", [128, 4, AD])          # dec_att_w.T [dd, a]
    cvwT_d = din("cvwT", [128, 4, CD])            # conv_w.T [p(pad 512), cd]
    cawT_d = din("cawT", [128, 2, AD])            # conv_att_w.T [cd, a]
    eawT_d = din("eawT", [128, 4, AD])            # enc_att_w.T [ed, a]
    wsel_d = din("wsel", [128, 2, BL, BL])        # full_att_w selection columns
    s0_2d_d = din("s0_2d", [B, DD])
    s0T_d = din("s0T", [128, 4, B])
    emb0T_d = din("emb0T", [128, 2, B])
    eabT_d = din("eabT", [128, 2])
    ab0T_d = din("ab0T", [128, 2])
    ab1T_d = din("ab1T", [128, 2])
    Sown_d = din("Sown", [B, BL])
    vofff_d = din("vofff", [B, 1])
    maskpT_d = din("maskpT", [B, Tmax])
    maskaT_d = din("maskaT", [BL, Tmax])
    embed_d = din("embed", [V, EMB])

    preds_o = nc.dram_tensor("preds", [B, Tmax, VL], F32, kind="ExternalOutput")
    alphas_o = nc.dram_tensor("alphas", [BL, Tmax, P], F32, kind="ExternalOutput")

    with tile.TileContext(nc) as tc:
        with (
            tc.tile_pool(name="cst", bufs=1) as cst,
            tc.tile_pool(name="strm", bufs=2) as strm,
            tc.tile_pool(name="att1s", bufs=2) as att1p,
            tc.tile_pool(name="wk", bufs=1) as wk,
            tc.tile_pool(name="ps", bufs=1, space="PSUM") as ps,
            tc.tile_pool(name="dram", bufs=1, space="DRAM") as dram,
        ):
            # ---------- resident tensors ----------
            encP = cst.tile([128, 4, BL, ED], F32)
            nc.sync.dma_start(encP[:], encP_d[:])
            fcwT = cst.tile([128, 4, VL], F32)
            nc.sync.dma_start(fcwT[:], fcwT_d[:])
            datwT = cst.tile([128, 4, AD], F32)
            nc.sync.dma_start(datwT[:], datwT_d[:])
            cvwT = cst.tile([128, 4, CD], F32)
            nc.sync.dma_start(cvwT[:], cvwT_d[:])
            cawT = cst.tile([128, 2, AD], F32)
            nc.sync.dma_start(cawT[:], cawT_d[:])
            wsel = cst.tile([128, 2, BL, BL], F32)
            nc.sync.dma_start(wsel[:], wsel_d[:])
            eabT = cst.tile([128, 2], F32)
            nc.sync.dma_start(eabT[:], eabT_d[:])
            ab0T = cst.tile([128, 2], F32)
            nc.sync.dma_start(ab0T[:], ab0T_d[:])
            ab1T = cst.tile([128, 2], F32)
            nc.sync.dma_start(ab1T[:], ab1T_d[:])
            Sown = cst.tile([B, BL], F32)
            nc.sync.dma_start(Sown[:], Sown_d[:])
            vofff = cst.tile([B, 1], F32)
            nc.sync.dma_start(vofff[:], vofff_d[:])
            maskpT = cst.tile([B, Tmax], F32)
            nc.sync.dma_start(maskpT[:], maskpT_d[:])
            maskaT = cst.tile([BL, Tmax], F32)
            nc.sync.dma_start(maskaT[:], maskaT_d[:])
            ident = cst.tile([128, 128], F32)
            make_identity(nc, ident[:])
            bigt = cst.tile([B, NT * 8], F32)
            nc.vector.memset(bigt[:], BIG)

            st2d = [cst.tile([B, DD], F32, name=f"st2d{i}") for i in range(2)]
            stT = [cst.tile([128, 4, B], F32, name=f"stT{i}") for i in range(2)]
            embT = [cst.tile([128, 2, B], F32, name=f"embT{i}") for i in range(2)]
            cvT = cst.tile([128, 4, BL], F32)
            alphaT_sb = cst.tile([128, 4, BL], F32)
            alphasel = cst.tile([128, 4, BL, BL], F32)
            nc.vector.memset(cvT[:], 0.0)
            nc.vector.memset(alphaT_sb[:], 0.0)
            nc.vector.memset(alphasel[:], 0.0)
            nc.sync.dma_start(st2d[0][:], s0_2d_d[:])
            nc.sync.dma_start(stT[0][:], s0T_d[:])
            nc.sync.dma_start(embT[0][:], emb0T_d[:])

            att1_h = dram.tile([BL, 2, 128, P], F32)
            agc_in = dram.tile([BL, DD], F32)
            agc_out = dram.tile([B, DD], F32)
            agk_in = dram.tile([B, 2], F32)
            agk_out = dram.tile([NC * B, 2], F32)
            groups = [list(range(NC))]

            # psum tags: fc(2x2K) gg2(2x2K) t64(1x2K) tsm(1x2K) tny(2x2K) = 16K
            def ps_fc():
                return ps.tile([B, VT], F32, tag="fc", bufs=2, name="psfc")

            def ps_gg(shape, name):
                return ps.tile(shape, F32, tag="gg2", bufs=2, name=name)

            def ps_t64(shape, name):
                return ps.tile(shape, F32, tag="t64", bufs=1, name=name)

            def ps_tsm(shape, name):
                return ps.tile(shape, F32, tag="tsm", bufs=1, name=name)

            def ps_tny(shape, name):
                return ps.tile(shape, F32, tag="tny", bufs=2, name=name)

            # ---------- init: att1 = enc @ enc_att_w.T + enc_att_b ----------
            eaw = strm.tile([128, 4, AD], F32, tag="wst", name="eaw")
            nc.sync.dma_start(eaw[:], eawT_d[:])
            for b in range(BL):
                a1p = [ps_gg([128, ED], f"a1p{ac}") for ac in range(2)]
                for dc in range(4):
                    tp = ps_tsm([128, ED], "initT")
                    for pc in range(4):
                        nc.tensor.transpose(
                            tp[:, pc * 128:(pc + 1) * 128],
                            encP[:, pc, b, dc * 128:(dc + 1) * 128],
                            ident[:])
                    et = wk.tile([128, ED], F32, tag="encT", bufs=2, name="et")
                    nc.scalar.copy(et[:], tp[:])
                    for ac in range(2):
                        nc.tensor.matmul(
                            a1p[ac][:],
                            eaw[:, dc, ac * 128:(ac + 1) * 128],
                            et[:], start=(dc == 0), stop=(dc == 3))
                for ac in range(2):
                    a1s = wk.tile([128, ED], F32, tag="a1s", bufs=1, name="a1s")
                    nc.scalar.activation(a1s[:], a1p[ac][:], AF.Identity,
                                         bias=eabT[:, ac:ac + 1], scale=1.0)
                    nc.sync.dma_start(att1_h[b, ac], a1s[:, :P])

            # ---------- decode loop ----------
            for t in range(Tmax):
                cur, nxt = t % 2, (t + 1) % 2
                sT = stT[cur]
                s2 = st2d[cur]
                eT = embT[cur]
                last = (t == Tmax - 1)

                # ===== B. attention on own rows, uses s_{t-1} =====
                sop = ps_tsm([BL, DD], "sop")
                nc.tensor.matmul(sop[:], Sown[:], s2[:], start=True, stop=True)
                so = wk.tile([BL, DD], F32, tag="so", name="so")
                nc.scalar.copy(so[:], sop[:])
                soTp = ps_tny([128, 4, BL], "soTp")
                for kc in range(4):
                    nc.tensor.transpose(soTp[:, kc, :],
                                        so[:, kc * 128:(kc + 1) * 128],
                                        ident[:BL, :BL])
                soT = wk.tile([128, 4, BL], F32, tag="soT", name="soT")
                nc.vector.tensor_copy(soT[:], soTp[:])
                a2p = ps_tsm([BL, AD], "a2p")
                for kc in range(4):
                    nc.tensor.matmul(a2p[:], soT[:, kc, :], datwT[:, kc, :],
                                     start=(kc == 0), stop=(kc == 3))
                a2 = wk.tile([BL, AD], F32, tag="a2", name="a2")
                nc.scalar.copy(a2[:], a2p[:])
                a23p = ps_tny([128, 2, BL], "a23p")
                for ac in range(2):
                    nc.tensor.transpose(a23p[:, ac, :],
                                        a2[:, ac * 128:(ac + 1) * 128],
                                        ident[:BL, :BL])
                if t >= 1:
                    cp = ps_tsm([BL, CD], "cp")
                    for pc in range(4):
                        nc.tensor.matmul(cp[:], cvT[:, pc, :], cvwT[:, pc, :],
                                         start=(pc == 0), stop=(pc == 3))
                    csb = wk.tile([BL, CD], F32, tag="csb", name="csb")
                    nc.scalar.copy(csb[:], cp[:])
                    cTp = ps_tny([128, 2, BL], "cTp")
                    for kc in range(2):
                        nc.tensor.transpose(cTp[:, kc, :],
                                            csb[:, kc * 128:(kc + 1) * 128],
                                            ident[:BL, :BL])
                    cT = wk.tile([128, 2, BL], F32, tag="cT", name="cT")
                    nc.vector.tensor_copy(cT[:], cTp[:])
                    for ac in range(2):
                        for kc in range(2):
                            nc.tensor.matmul(
                                a23p[:, ac, :],
                                cawT[:, kc, ac * 128:(ac + 1) * 128],
                                cT[:, kc, :], start=False, stop=(kc == 1))
                a23 = wk.tile([128, 2, BL], F32, tag="a23", name="a23")
                abT = ab1T if t >= 1 else ab0T
                for ac in range(2):
                    nc.scalar.activation(a23[:, ac, :], a23p[:, ac, :],
                                         AF.Identity, bias=abT[:, ac:ac + 1],
                                         scale=1.0)
                scp = ps_tsm([BL, P], "scp")
                for b in range(BL):
                    a1t = att1p.tile([128, 2, P], F32, tag="a1t", name="a1t")
                    nc.sync.dma_start(
                        a1t[:], att1_h[b].rearrange("a k p -> k a p"))
                    for ac in range(2):
                        et2 = wk.tile([128, P], F32, tag="et2", bufs=1, name="et2")
                        nc.scalar.activation(et2[:], a1t[:, ac, :], AF.Tanh,
                                             bias=a23[:, ac, b:b + 1], scale=1.0)
                        nc.tensor.matmul(scp[:], wsel[:, ac, b, :], et2[:],
                                         start=(b == 0 and ac == 0),
                                         stop=(b == BL - 1 and ac == 1))
                nmax = wk.tile([BL, 1], F32, tag="nmax", name="nmax")
                nc.vector.tensor_reduce(nmax[:], scp[:], axis=AX.X, op=OP.max)
                nc.vector.tensor_scalar_mul(nmax[:], nmax[:], -1.0)
                exps = wk.tile([BL, P], F32, tag="exps", name="exps")
                sume = wk.tile([BL, 1], F32, tag="sume", name="sume")
                nc.scalar.activation(exps[:], scp[:], AF.Exp, bias=nmax[:],
                                     scale=1.0, accum_out=sume[:])
                rsum = wk.tile([BL, 1], F32, tag="rsum", name="rsum")
                nc.vector.reciprocal(rsum[:], sume[:])
                nc.vector.tensor_scalar_mul(exps[:], exps[:], rsum[:])  # alpha
                aout = wk.tile([BL, P], F32, tag="aout", name="aout")
                nc.scalar.activation(aout[:], exps[:], AF.Copy,
                                     scale=maskaT[:, t:t + 1])
                nc.sync.dma_start(alphas_o[:, t, :], aout[:])
                atp = ps_tny([128, 4, BL], "atp")
                for pc in range(3):
                    nc.tensor.transpose(atp[:, pc, :],
                                        exps[:, pc * 128:(pc + 1) * 128],
                                        ident[:BL, :BL])
                nc.tensor.transpose(atp[:16, 3, :], exps[:, 384:400],
                                    ident[:BL, :BL])
                for pc in range(3):
                    nc.vector.tensor_copy(alphaT_sb[:, pc, :], atp[:, pc, :])
                    nc.vector.tensor_copy(
                        alphasel[:, pc].rearrange("k a b -> k (a b)")[:, 0:64:9],
                        atp[:, pc, :])
                nc.vector.tensor_copy(alphaT_sb[:16, 3, :], atp[:16, 3, :])
                nc.vector.tensor_copy(
                    alphasel[:16, 3].rearrange("k a b -> k (a b)")[:, 0:64:9],
                    atp[:16, 3, :])
                ctxp = ps_tsm([BL, ED], "ctxp")
                for b in range(BL):
                    for pc in range(4):
                        nc.tensor.matmul(
                            ctxp[:], alphasel[:, pc, b, :],
                            encP[:, pc, b, :],
                            start=(b == 0 and pc == 0),
                            stop=(b == BL - 1 and pc == 3))
                ctx = wk.tile([BL, ED], F32, tag="ctx", name="ctx")
                nc.scalar.copy(ctx[:], ctxp[:])
                nc.vector.tensor_add(cvT[:], cvT[:], alphaT_sb[:])
                nc.sync.dma_start(agc_in[:], ctx[:])
                nc.gpsimd.collective_compute(
                    "AllGather", OP.bypass, replica_groups=groups,
                    ins=[agc_in[:].opt()], outs=[agc_out[:].opt()])
                ctx2 = wk.tile([B, ED], F32, tag="ctx2", name="ctx2")
                nc.sync.dma_start(ctx2[:], agc_out[:])
                cxTp = ps_t64([128, 4, B], "cxTp")
                for kc in range(4):
                    nc.tensor.transpose(cxTp[:, kc, :],
                                        ctx2[:, kc * 128:(kc + 1) * 128],
                                        ident[:B, :B])
                ctxT = wk.tile([128, 4, B], F32, tag="ctxT", name="ctxT")
                nc.vector.tensor_copy(ctxT[:], cxTp[:])

                # ===== C/D. the two GRU cells (full batch) =====
                def gru(xT, kx, wih_d_, hT, h2, whh_d_, name):
                    def wload(wd, kn, g, sfx):
                        wt = strm.tile([128, kn, DD], F32, tag="wst",
                                       name=f"w{name}{sfx}")
                        nc.sync.dma_start(wt[:],
                                          wd[:, :, g * DD:(g + 1) * DD])
                        return wt

                    def gate_sum(g, gname):
                        # gi + gh accumulated in one psum tile by the matmuls
                        wi = wload(wih_d_, kx, g, f"i{g}")
                        wh = wload(whh_d_, 4, g, f"h{g}")
                        gp = ps_gg([B, DD], gname)
                        for k in range(kx):
                            nc.tensor.matmul(gp[:], xT[:, k, :], wi[:, k, :],
                                             start=(k == 0), stop=False)
                        for k in range(4):
                            nc.tensor.matmul(gp[:], hT[:, k, :], wh[:, k, :],
                                             start=False, stop=(k == 3))
                        return gp

                    # r gate
                    gp = gate_sum(0, f"gr{name}")
                    r = wk.tile([B, DD], F32, tag="rga", name="rga")
                    nc.scalar.activation(r[:], gp[:], AF.Sigmoid)
                    # n gate (separate i/h parts; keep r alive)
                    wi = wload(wih_d_, kx, 2, "i2")
                    gpi = ps_gg([B, DD], f"gni{name}")
                    for k in range(kx):
                        nc.tensor.matmul(gpi[:], xT[:, k, :], wi[:, k, :],
                                         start=(k == 0), stop=(k == kx - 1))
                    wh = wload(whh_d_, 4, 2, "h2")
                    gph = ps_gg([B, DD], f"gnh{name}")
                    for k in range(4):
                        nc.tensor.matmul(gph[:], hT[:, k, :], wh[:, k, :],
                                         start=(k == 0), stop=(k == 3))
                    n = wk.tile([B, DD], F32, tag="nga", name="nga")
                    nc.vector.tensor_tensor(n[:], r[:], gph[:], op=OP.mult)
                    nc.vector.tensor_add(n[:], n[:], gpi[:])
                    nc.scalar.activation(n[:], n[:], AF.Tanh)
                    # z gate
                    gp = gate_sum(1, f"gz{name}")
                    z = wk.tile([B, DD], F32, tag="zga", name="zga")
                    nc.scalar.activation(z[:], gp[:], AF.Sigmoid)
                    # out = (h - n) * z + n
                    nc.vector.tensor_tensor(r[:], h2[:], n[:], op=OP.subtract)
                    nc.vector.tensor_tensor(r[:], r[:], z[:], op=OP.mult)
                    out2 = wk.tile([B, DD], F32, tag=f"o2{name}", name=f"o2{name}")
                    nc.vector.tensor_add(out2[:], r[:], n[:])
                    oTp = ps_t64([128, 4, B], f"oT{name}")
                    for kc in range(4):
                        nc.tensor.transpose(oTp[:, kc, :],
                                            out2[:, kc * 128:(kc + 1) * 128],
                                            ident[:B, :B])
                    return out2, oTp

                sh2, shTp = gru(eT, 2, wih1_d, sT, s2, whh1_d, "1")
                shT = wk.tile([128, 4, B], F32, tag="shT", name="shT")
                nc.vector.tensor_copy(shT[:], shTp[:])
                new2, newTp = gru(ctxT, 4, wih2_d, shT, sh2, whh2_d, "2")
                nc.vector.tensor_copy(st2d[nxt][:], new2[:])
                nc.scalar.copy(stT[nxt][:], newTp[:])

                # ===== A. vocab projection + greedy argmax feedback =====
                svals = wk.tile([B, NT * 8], F32, tag="svals", name="svals")
                sidxf = wk.tile([B, NT * 8], F32, tag="sidxf", name="sidxf")
                for nt in range(NT):
                    pt = ps_fc()
                    for k in range(4):
                        nc.tensor.matmul(
                            pt[:], stT[nxt][:, k, :], fcwT[:, k, nt * VT:(nt + 1) * VT],
                            start=(k == 0), stop=(k == 3))
                    pr = wk.tile([B, VT], F32, tag="pr", bufs=2, name="pr")
                    if nt % 2 == 0:
                        nc.scalar.activation(pr[:], pt[:], AF.Copy,
                                             scale=maskpT[:, t:t + 1])
                    else:
                        nc.vector.tensor_scalar_mul(pr[:], pt[:],
                                                    maskpT[:, t:t + 1])
                    nc.sync.dma_start(preds_o[:, t, nt * VT:(nt + 1) * VT], pr[:])
                    if not last:
                        sl = slice(nt * 8, (nt + 1) * 8)
                        nc.vector.max(svals[:, sl], pr[:])
                        mi8 = wk.tile([B, 8], U32, tag="mi8", name="mi8")
                        nc.vector.max_index(mi8[:], svals[:, sl], pr[:])
                        nc.vector.tensor_copy(sidxf[:, sl], mi8[:])   # u32->f32
                        nc.vector.tensor_scalar_add(sidxf[:, sl], sidxf[:, sl],
                                                    float(nt * VT))
                if not last:
                    # within-core top-1 (value max, tie -> smallest index)
                    gmax = wk.tile([B, 1], F32, tag="gmax", name="gmax")
                    nc.vector.tensor_reduce(gmax[:], svals[:], axis=AX.X, op=OP.max)
                    meq = wk.tile([B, NT * 8], U8, tag="meq", name="meq")
                    nc.vector.tensor_scalar(meq[:], svals[:], gmax[:], None,
                                            op0=OP.is_equal)
                    seli = wk.tile([B, NT * 8], F32, tag="seli", name="seli")
                    nc.vector.select(seli[:], meq[:], sidxf[:], bigt[:])
                    lidx = wk.tile([B, 1], F32, tag="lidx", name="lidx")
                    nc.vector.tensor_reduce(lidx[:], seli[:], axis=AX.X, op=OP.min)
                    cand = wk.tile([B, 2], F32, tag="cand", name="cand")
                    nc.vector.tensor_copy(cand[:, 0:1], gmax[:])
                    nc.vector.tensor_tensor(cand[:, 1:2], lidx[:], vofff[:],
                                            op=OP.add)
                    nc.sync.dma_start(agk_in[:], cand[:])
                    nc.gpsimd.collective_compute(
                        "AllGather", OP.bypass, replica_groups=groups,
                        ins=[agk_in[:].opt()], outs=[agk_out[:].opt()])
                    call = wk.tile([B, NC, 2], F32, tag="call", name="call")
                    nc.sync.dma_start(
                        call[:], agk_out[:].rearrange("(c b) k -> b c k", c=NC))
                    g2 = wk.tile([B, 1], F32, tag="g2", name="g2")
                    nc.vector.tensor_reduce(g2[:], call[:, :, 0], axis=AX.X,
                                            op=OP.max)
                    m2 = wk.tile([B, NC], U8, tag="m2", name="m2")
                    nc.vector.tensor_scalar(m2[:], call[:, :, 0], g2[:], None,
                                            op0=OP.is_equal)
                    s2i = wk.tile([B, NC], F32, tag="s2i", name="s2i")
                    nc.vector.select(s2i[:], m2[:], call[:, :, 1], bigt[:, :NC])
                    wf = wk.tile([B, 1], F32, tag="wf", name="wf")
                    nc.vector.tensor_reduce(wf[:], s2i[:], axis=AX.X, op=OP.min)
                    wu = wk.tile([B, 1], U32, tag="wu", name="wu")
                    nc.vector.tensor_copy(wu[:], wf[:])
                    embn = wk.tile([B, EMB], F32, tag="embn", name="embn")
                    nc.gpsimd.indirect_dma_start(
                        out=embn[:], out_offset=None, in_=embed_d[:],
                        in_offset=bass.IndirectOffsetOnAxis(ap=wu[:, :1], axis=0))
                    etp = ps_t64([128, 2, B], "etp")
                    for kc in range(2):
                        nc.tensor.transpose(
                            etp[:, kc, :], embn[:, kc * 128:(kc + 1) * 128],
                            ident[:B, :B])
                    nc.scalar.copy(embT[nxt][:], etp[:])


    nc.finalize()
    return nc


_CACHE = {}


def _get_nc(Tmax):
    if Tmax not in _CACHE:
        _CACHE[Tmax] = build(Tmax)
    return _CACHE[Tmax]


def _chunkT(w, kchunks):
    """[out, in] weight -> lhsT/rhs SBUF layout [128, kchunks, out]."""
    o, i = w.shape
    assert i == kchunks * 128
    return np.ascontiguousarray(
        w.T.reshape(kchunks, 128, o).transpose(1, 0, 2)).astype(np.float32)


def kernel(encoder_out, encoded_captions, caption_lengths, embedding,
           gru1_w_ih, gru1_w_hh, gru1_b_ih, gru1_b_hh,
           gru2_w_ih, gru2_w_hh, gru2_b_ih, gru2_b_hh,
           enc_att_w, enc_att_b, dec_att_w, dec_att_b,
           conv_w, conv_b, conv_att_w, conv_att_b,
           full_att_w, full_att_b, s_w, s_b, fc_w, fc_b, _trace=False):
    encoder_out = np.asarray(encoder_out, np.float32)
    caption_lengths = np.asarray(caption_lengths)
    encoded_captions = np.asarray(encoded_captions)
    embedding = np.asarray(embedding, np.float32)
    f32 = lambda x: np.asarray(x, np.float32)
    gru1_w_ih, gru1_w_hh, gru2_w_ih, gru2_w_hh = map(
        f32, (gru1_w_ih, gru1_w_hh, gru2_w_ih, gru2_w_hh))
    enc_att_w, dec_att_w, conv_w, conv_att_w, full_att_w, s_w, fc_w = map(
        f32, (enc_att_w, dec_att_w, conv_w, conv_att_w, full_att_w, s_w, fc_w))
    for nm, bb in (("gru biases", (gru1_b_ih, gru1_b_hh, gru2_b_ih, gru2_b_hh)),
                   ("fc_b", (fc_b,))):
        for x in bb:
            assert not np.any(np.asarray(x)), f"nonzero {nm} not supported"

    sort_ind = np.argsort(-caption_lengths, kind="stable")
    lens = caption_lengths[sort_ind]
    enc = encoder_out[sort_ind]
    caps = encoded_captions[sort_ind]
    Tmax = int(lens.max())

    mean = enc.mean(axis=1, dtype=np.float32).astype(np.float32)
    s0 = (mean @ s_w.T + np.asarray(s_b, np.float32)).astype(np.float32)
    s0T = _chunkT(s0, 4)
    emb0 = np.ascontiguousarray(embedding[np.asarray(caps[:, 0], np.int64)])
    emb0T = _chunkT(emb0, 2)

    wih1 = _chunkT(gru1_w_ih, 2)
    whh1 = _chunkT(gru1_w_hh, 4)
    wih2 = _chunkT(gru2_w_ih, 4)
    whh2 = _chunkT(gru2_w_hh, 4)
    datwT = _chunkT(dec_att_w, 4)
    cawT = _chunkT(conv_att_w, 2)
    eawT = _chunkT(enc_att_w, 4)
    conv_w_pad = np.zeros((CD, 512), np.float32)
    conv_w_pad[:, :P] = conv_w
    cvwT = _chunkT(conv_w_pad, 4)
    eabT = np.ascontiguousarray(np.asarray(enc_att_b, np.float32).reshape(2, 128).T)
    ab0 = np.asarray(dec_att_b, np.float32)
    ab1 = (ab0 + np.asarray(conv_att_b, np.float32)
           + conv_att_w @ np.asarray(conv_b, np.float32)).astype(np.float32)
    ab0T = np.ascontiguousarray(ab0.reshape(2, 128).T)
    ab1T = np.ascontiguousarray(ab1.reshape(2, 128).T)
    wv = full_att_w[0].astype(np.float32)
    wsel = np.zeros((128, 2, BL, BL), np.float32)
    for ac in range(2):
        for b in range(BL):
            wsel[:, ac, b, b] = wv[ac * 128:(ac + 1) * 128]
    maskpT = (np.arange(Tmax)[None, :] < np.asarray(lens)[:, None]).astype(np.float32)
    maskpT = np.ascontiguousarray(maskpT)

    nc = _get_nc(Tmax)

    in_maps = []
    for c in range(NC):
        rows = slice(c * BL, (c + 1) * BL)
        enc_own = enc[rows]
        encP = np.zeros((128, 4, BL, ED), np.float32)
        ep = enc_own.transpose(1, 0, 2)   # [p, b, d]
        for pc in range(4):
            n = min(128, P - pc * 128)
            encP[:n, pc] = ep[pc * 128:pc * 128 + n]
        fcwT = _chunkT(fc_w[c * VL:(c + 1) * VL], 4)
        Sown = np.zeros((B, BL), np.float32)
        for j in range(BL):
            Sown[c * BL + j, j] = 1.0
        in_maps.append(dict(
            encP=encP, fcwT=fcwT, wih1=wih1, whh1=whh1, wih2=wih2, whh2=whh2,
            datwT=datwT, cvwT=cvwT, cawT=cawT, eawT=eawT, wsel=wsel,
            s0_2d=s0, s0T=s0T, emb0T=emb0T, eabT=eabT, ab0T=ab0T, ab1T=ab1T,
            Sown=Sown,
            vofff=np.full((B, 1), float(c * VL), np.float32),
            maskpT=maskpT,
            maskaT=np.ascontiguousarray(maskpT[rows]),
            embed=embedding,
        ))

    try:
        res = run_bass_kernel_spmd(nc, in_maps, core_ids=list(range(NC)),
                                   trace=_trace)
    except ModuleNotFoundError:
        # axon NTFF profile hook unavailable in this environment
        os.environ["BASS_NEVER_TRACE"] = "1"
        res = run_bass_kernel_spmd(nc, in_maps, core_ids=list(range(NC)),
                                   trace=False)
    predictions = np.concatenate(
        [res.results[c]["preds"] for c in range(NC)], axis=2)
    alphas = np.concatenate(
        [res.results[c]["alphas"] for c in range(NC)], axis=0)

    out = (predictions, caps, lens, alphas, sort_ind.astype(np.int32))
    if _trace:
        return out, res
    return out


# revision 11
# speedup vs baseline: 1.8193x; 1.0506x over previous
"""Trainium2 Bass kernel for nn_Decoder (show-attend-tell greedy decoder).

Sharding across 8 NeuronCores:
  - attention path batch-sharded (8 rows/core, enc slice resident in SBUF)
  - GRU state path replicated (full batch 64; matmul cost is N-bound so
    replication costs nothing and avoids an extra collective)
  - vocab projection sharded column-wise (fc_w.T slice [512,4000] resident in
    SBUF per core); exact fp32 greedy argmax via AllGather of per-core top-1.
Per step: AllGather(context [8,512] -> [64,512]) and AllGather(argmax
candidates [64,2] -> [8,64,2]).
"""
import os

import numpy as np

import concourse.bass as bass
import concourse.mybir as mybir
import concourse.tile as tile
from concourse import bacc
from concourse.bass_utils import run_bass_kernel_spmd
from concourse.masks import make_identity

F32 = mybir.dt.float32
U32 = mybir.dt.uint32
U8 = mybir.dt.uint8
AF = mybir.ActivationFunctionType
OP = mybir.AluOpType
AX = mybir.AxisListType

NC = 8           # cores
B = 64           # batch
BL = B // NC     # batch rows per core
P = 400          # pixels
ED = 512         # encoder dim
DD = 512         # decoder dim
EMB = 256        # embedding dim
AD = 256         # attention dim
CD = 256         # conv dim
V = 32000        # vocab
VL = V // NC     # vocab slice per core
NT = 8           # fc n-tiles
VT = VL // NT    # 500
BIG = 1.0e9


def build(Tmax: int):
    nc = bacc.Bacc(num_devices=NC)

    def din(name, shape, dt=F32):
        return nc.dram_tensor(name, shape, dt, kind="ExternalInput")

    encP_d = din("encP", [128, 4, BL, ED])        # [ki, pc, b, d]; p=pc*128+ki, zero-padded
    fcwT_d = din("fcwT", [128, 4, VL])            # fc_w slice .T  [dd, v]
    wih1_d = din("wih1", [128, 2, 3 * DD])        # gru1 w_ih.T  [emb, 3dd]
    whh1_d = din("whh1", [128, 4, 3 * DD])
    wih2_d = din("wih2", [128, 4, 3 * DD])
    whh2_d = din("whh2", [128, 4, 3 * DD])
    datwT_d = din("datwT", [128, 4, AD])          # dec_att_w.T [dd, a]
    cvwT_d = din("cvwT", [128, 4, CD])            # conv_w.T [p(pad 512), cd]
    cawT_d = din("cawT", [128, 2, AD])            # conv_att_w.T [cd, a]
    eawT_d = din("eawT", [128, 4, AD])            # enc_att_w.T [ed, a]
    wsel_d = din("wsel", [128, 2, BL, BL])        # full_att_w selection columns
    s0_2d_d = din("s0_2d", [B, DD])
    s0T_d = din("s0T", [128, 4, B])
    emb0T_d = din("emb0T", [128, 2, B])
    eabT_d = din("eabT", [128, 2])
    ab0T_d = din("ab0T", [128, 2])
    ab1T_d = din("ab1T", [128, 2])
    Sown_d = din("Sown", [B, BL])
    vofff_d = din("vofff", [B, 1])
    maskpT_d = din("maskpT", [B, Tmax])
    maskaT_d = din("maskaT", [BL, Tmax])
    embed_d = din("embed", [V, EMB])

    preds_o = nc.dram_tensor("preds", [B, Tmax, VL], F32, kind="ExternalOutput")
    alphas_o = nc.dram_tensor("alphas", [BL, Tmax, P], F32, kind="ExternalOutput")

    with tile.TileContext(nc) as tc:
        with (
            tc.tile_pool(name="cst", bufs=1) as cst,
            tc.tile_pool(name="strm", bufs=2) as strm,
            tc.tile_pool(name="att1s", bufs=2) as att1p,
            tc.tile_pool(name="wk", bufs=1) as wk,
            tc.tile_pool(name="ps", bufs=1, space="PSUM") as ps,
            tc.tile_pool(name="dram", bufs=1, space="DRAM") as dram,
        ):
            # ---------- resident tensors ----------
            encP = cst.tile([128, 4, BL, ED], F32)
            nc.sync.dma_start(encP[:], encP_d[:])
            fcwT = cst.tile([128, 4, VL], F32)
            nc.sync.dma_start(fcwT[:], fcwT_d[:])
            datwT = cst.tile([128, 4, AD], F32)
            nc.sync.dma_start(datwT[:], datwT_d[:])
            cvwT = cst.tile([128, 4, CD], F32)
            nc.sync.dma_start(cvwT[:], cvwT_d[:])
            cawT = cst.tile([128, 2, AD], F32)
            nc.sync.dma_start(cawT[:], cawT_d[:])
            wsel = cst.tile([128, 2, BL, BL], F32)
            nc.sync.dma_start(wsel[:], wsel_d[:])
            eabT = cst.tile([128, 2], F32)
            nc.sync.dma_start(eabT[:], eabT_d[:])
            ab0T = cst.tile([128, 2], F32)
            nc.sync.dma_start(ab0T[:], ab0T_d[:])
            ab1T = cst.tile([128, 2], F32)
            nc.sync.dma_start(ab1T[:], ab1T_d[:])
            Sown = cst.tile([B, BL], F32)
            nc.sync.dma_start(Sown[:], Sown_d[:])
            vofff = cst.tile([B, 1], F32)
            nc.sync.dma_start(vofff[:], vofff_d[:])
            maskpT = cst.tile([B, Tmax], F32)
            nc.sync.dma_start(maskpT[:], maskpT_d[:])
            maskaT = cst.tile([BL, Tmax], F32)
            nc.sync.dma_start(maskaT[:], maskaT_d[:])
            ident = cst.tile([128, 128], F32)
            make_identity(nc, ident[:])
            bigt = cst.tile([B, NT * 8], F32)
            nc.vector.memset(bigt[:], BIG)

            st2d = [cst.tile([B, DD], F32, name=f"st2d{i}") for i in range(2)]
            stT = [cst.tile([128, 4, B], F32, name=f"stT{i}") for i in range(2)]
            embT = [cst.tile([128, 2, B], F32, name=f"embT{i}") for i in range(2)]
            cvT = cst.tile([128, 4, BL], F32)
            alphaT_sb = cst.tile([128, 4, BL], F32)
            alphasel = cst.tile([128, 4, BL, BL], F32)
            nc.vector.memset(cvT[:], 0.0)
            nc.vector.memset(alphaT_sb[:], 0.0)
            nc.vector.memset(alphasel[:], 0.0)
            nc.sync.dma_start(st2d[0][:], s0_2d_d[:])
            nc.sync.dma_start(stT[0][:], s0T_d[:])
            nc.sync.dma_start(embT[0][:], emb0T_d[:])

            att1_h = dram.tile([BL, 2, 128, P], F32)
            agc_in = dram.tile([BL, DD], F32)
            agc_out = dram.tile([B, DD], F32)
            agk_in = dram.tile([B, 2], F32)
            agk_out = dram.tile([NC * B, 2], F32)
            groups = [list(range(NC))]

            # psum tags: fc(2x2K) gg2(2x2K) t64(1x2K) tsm(1x2K) tny(2x2K) = 16K
            def ps_fc():
                return ps.tile([B, VT], F32, tag="fc", bufs=2, name="psfc")

            def ps_gg(shape, name):
                return ps.tile(shape, F32, tag="gg2", bufs=2, name=name)

            def ps_t64(shape, name):
                return ps.tile(shape, F32, tag="t64", bufs=1, name=name)

            def ps_tsm(shape, name):
                return ps.tile(shape, F32, tag="tsm", bufs=1, name=name)

            def ps_tny(shape, name):
                return ps.tile(shape, F32, tag="tny", bufs=2, name=name)

            # ---------- init: att1 = enc @ enc_att_w.T + enc_att_b ----------
            eaw = strm.tile([128, 4, AD], F32, tag="wst", name="eaw")
            nc.sync.dma_start(eaw[:], eawT_d[:])
            for b in range(BL):
                a1p = [ps_gg([128, ED], f"a1p{ac}") for ac in range(2)]
                for dc in range(4):
                    tp = ps_tsm([128, ED], "initT")
                    for pc in range(4):
                        nc.tensor.transpose(
                            tp[:, pc * 128:(pc + 1) * 128],
                            encP[:, pc, b, dc * 128:(dc + 1) * 128],
                            ident[:])
                    et = wk.tile([128, ED], F32, tag="encT", bufs=2, name="et")
                    nc.scalar.copy(et[:], tp[:])
                    for ac in range(2):
                        nc.tensor.matmul(
                            a1p[ac][:],
                            eaw[:, dc, ac * 128:(ac + 1) * 128],
                            et[:], start=(dc == 0), stop=(dc == 3))
                for ac in range(2):
                    a1s = wk.tile([128, ED], F32, tag="a1s", bufs=1, name="a1s")
                    nc.scalar.activation(a1s[:], a1p[ac][:], AF.Identity,
                                         bias=eabT[:, ac:ac + 1], scale=1.0)
                    nc.sync.dma_start(att1_h[b, ac], a1s[:, :P])

            # ---------- decode loop ----------
            for t in range(Tmax):
                cur, nxt = t % 2, (t + 1) % 2
                sT = stT[cur]
                s2 = st2d[cur]
                eT = embT[cur]
                last = (t == Tmax - 1)

                # ===== B. attention on own rows, uses s_{t-1} =====
                sop = ps_tsm([BL, DD], "sop")
                nc.tensor.matmul(sop[:], Sown[:], s2[:], start=True, stop=True)
                so = wk.tile([BL, DD], F32, tag="so", name="so")
                nc.scalar.copy(so[:], sop[:])
                soTp = ps_tny([128, 4, BL], "soTp")
                for kc in range(4):
                    nc.tensor.transpose(soTp[:, kc, :],
                                        so[:, kc * 128:(kc + 1) * 128],
                                        ident[:BL, :BL])
                soT = wk.tile([128, 4, BL], F32, tag="soT", name="soT")
                nc.vector.tensor_copy(soT[:], soTp[:])
                a2p = ps_tsm([BL, AD], "a2p")
                for kc in range(4):
                    nc.tensor.matmul(a2p[:], soT[:, kc, :], datwT[:, kc, :],
                                     start=(kc == 0), stop=(kc == 3))
                a2 = wk.tile([BL, AD], F32, tag="a2", name="a2")
                nc.scalar.copy(a2[:], a2p[:])
                a23p = ps_tny([128, 2, BL], "a23p")
                for ac in range(2):
                    nc.tensor.transpose(a23p[:, ac, :],
                                        a2[:, ac * 128:(ac + 1) * 128],
                                        ident[:BL, :BL])
                if t >= 1:
                    cp = ps_tsm([BL, CD], "cp")
                    for pc in range(4):
                        nc.tensor.matmul(cp[:], cvT[:, pc, :], cvwT[:, pc, :],
                                         start=(pc == 0), stop=(pc == 3))
                    csb = wk.tile([BL, CD], F32, tag="csb", name="csb")
                    nc.scalar.copy(csb[:], cp[:])
                    cTp = ps_tny([128, 2, BL], "cTp")
                    for kc in range(2):
                        nc.tensor.transpose(cTp[:, kc, :],
                                            csb[:, kc * 128:(kc + 1) * 128],
                                            ident[:BL, :BL])
                    cT = wk.tile([128, 2, BL], F32, tag="cT", name="cT")
                    nc.vector.tensor_copy(cT[:], cTp[:])
                    for ac in range(2):
                        for kc in range(2):
                            nc.tensor.matmul(
                                a23p[:, ac, :],
                                cawT[:, kc, ac * 128:(ac + 1) * 128],
                                cT[:, kc, :], start=False, stop=(kc == 1))
                a23 = wk.tile([128, 2, BL], F32, tag="a23", name="a23")
                abT = ab1T if t >= 1 else ab0T
                for ac in range(2):
                    nc.scalar.activation(a23[:, ac, :], a23p[:, ac, :],
                                         AF.Identity, bias=abT[:, ac:ac + 1],
                                         scale=1.0)
                scp = ps_tsm([BL, P], "scp")
                for b in range(BL):
                    a1t = att1p.tile([128, 2, P], F32, tag="a1t", name="a1t")
                    nc.sync.dma_start(
                        a1t[:], att1_h[b].rearrange("a k p -> k a p"))
                    for ac in range(2):
                        et2 = wk.tile([128, P], F32, tag="et2", bufs=2, name="et2")
                        nc.scalar.activation(et2[:], a1t[:, ac, :], AF.Tanh,
                                             bias=a23[:, ac, b:b + 1], scale=1.0)
                        nc.tensor.matmul(scp[:], wsel[:, ac, b, :], et2[:],
                                         start=(b == 0 and ac == 0),
                                         stop=(b == BL - 1 and ac == 1))
                nmax = wk.tile([BL, 1], F32, tag="nmax", name="nmax")
                nc.vector.tensor_reduce(nmax[:], scp[:], axis=AX.X, op=OP.max)
                nc.vector.tensor_scalar_mul(nmax[:], nmax[:], -1.0)
                exps = wk.tile([BL, P], F32, tag="exps", name="exps")
                sume = wk.tile([BL, 1], F32, tag="sume", name="sume")
                nc.scalar.activation(exps[:], scp[:], AF.Exp, bias=nmax[:],
                                     scale=1.0, accum_out=sume[:])
                rsum = wk.tile([BL, 1], F32, tag="rsum", name="rsum")
                nc.vector.reciprocal(rsum[:], sume[:])
                nc.vector.tensor_scalar_mul(exps[:], exps[:], rsum[:])  # alpha
                aout = wk.tile([BL, P], F32, tag="aout", name="aout")
                nc.scalar.activation(aout[:], exps[:], AF.Copy,
                                     scale=maskaT[:, t:t + 1])
                nc.sync.dma_start(alphas_o[:, t, :], aout[:])
                atp = ps_tny([128, 4, BL], "atp")
                for pc in range(3):
                    nc.tensor.transpose(atp[:, pc, :],
                                        exps[:, pc * 128:(pc + 1) * 128],
                                        ident[:BL, :BL])
                nc.tensor.transpose(atp[:16, 3, :], exps[:, 384:400],
                                    ident[:BL, :BL])
                for pc in range(3):
                    nc.vector.tensor_copy(alphaT_sb[:, pc, :], atp[:, pc, :])
                    nc.vector.tensor_copy(
                        alphasel[:, pc].rearrange("k a b -> k (a b)")[:, 0:64:9],
                        atp[:, pc, :])
                nc.vector.tensor_copy(alphaT_sb[:16, 3, :], atp[:16, 3, :])
                nc.vector.tensor_copy(
                    alphasel[:16, 3].rearrange("k a b -> k (a b)")[:, 0:64:9],
                    atp[:16, 3, :])
                ctxp = ps_tsm([BL, ED], "ctxp")
                for b in range(BL):
                    for pc in range(4):
                        nc.tensor.matmul(
                            ctxp[:], alphasel[:, pc, b, :],
                            encP[:, pc, b, :],
                            start=(b == 0 and pc == 0),
                            stop=(b == BL - 1 and pc == 3))
                ctx = wk.tile([BL, ED], F32, tag="ctx", name="ctx")
                nc.scalar.copy(ctx[:], ctxp[:])
                nc.vector.tensor_add(cvT[:], cvT[:], alphaT_sb[:])
                nc.sync.dma_start(agc_in[:], ctx[:])
                nc.gpsimd.collective_compute(
                    "AllGather", OP.bypass, replica_groups=groups,
                    ins=[agc_in[:].opt()], outs=[agc_out[:].opt()])
                ctx2 = wk.tile([B, ED], F32, tag="ctx2", name="ctx2")
                nc.sync.dma_start(ctx2[:], agc_out[:])
                cxTp = ps_t64([128, 4, B], "cxTp")
                for kc in range(4):
                    nc.tensor.transpose(cxTp[:, kc, :],
                                        ctx2[:, kc * 128:(kc + 1) * 128],
                                        ident[:B, :B])
                ctxT = wk.tile([128, 4, B], F32, tag="ctxT", name="ctxT")
                nc.vector.tensor_copy(ctxT[:], cxTp[:])

                # ===== C/D. the two GRU cells (full batch) =====
                def gru(xT, kx, wih_d_, hT, h2, whh_d_, name):
                    def wload(wd, kn, g, sfx):
                        wt = strm.tile([128, kn, DD], F32, tag="wst",
                                       name=f"w{name}{sfx}")
                        nc.sync.dma_start(wt[:],
                                          wd[:, :, g * DD:(g + 1) * DD])
                        return wt

                    def gate_sum(g, gname):
                        # gi + gh accumulated in one psum tile by the matmuls
                        wi = wload(wih_d_, kx, g, f"i{g}")
                        wh = wload(whh_d_, 4, g, f"h{g}")
                        gp = ps_gg([B, DD], gname)
                        for k in range(kx):
                            nc.tensor.matmul(gp[:], xT[:, k, :], wi[:, k, :],
                                             start=(k == 0), stop=False)
                        for k in range(4):
                            nc.tensor.matmul(gp[:], hT[:, k, :], wh[:, k, :],
                                             start=False, stop=(k == 3))
                        return gp

                    # r gate
                    gp = gate_sum(0, f"gr{name}")
                    r = wk.tile([B, DD], F32, tag="rga", name="rga")
                    nc.scalar.activation(r[:], gp[:], AF.Sigmoid)
                    # n gate (separate i/h parts; keep r alive)
                    wi = wload(wih_d_, kx, 2, "i2")
                    gpi = ps_gg([B, DD], f"gni{name}")
                    for k in range(kx):
                        nc.tensor.matmul(gpi[:], xT[:, k, :], wi[:, k, :],
                                         start=(k == 0), stop=(k == kx - 1))
                    wh = wload(whh_d_, 4, 2, "h2")
                    gph = ps_gg([B, DD], f"gnh{name}")
                    for k in range(4):
                        nc.tensor.matmul(gph[:], hT[:, k, :], wh[:, k, :],
                                         start=(k == 0), stop=(k == 3))
                    n = wk.tile([B, DD], F32, tag="nga", name="nga")
                    nc.vector.tensor_tensor(n[:], r[:], gph[:], op=OP.mult)
                    nc.vector.tensor_add(n[:], n[:], gpi[:])
                    nc.scalar.activation(n[:], n[:], AF.Tanh)
                    # z gate
                    gp = gate_sum(1, f"gz{name}")
                    z = wk.tile([B, DD], F32, tag="zga", name="zga")
                    nc.scalar.activation(z[:], gp[:], AF.Sigmoid)
                    # out = (h - n) * z + n
                    nc.vector.tensor_tensor(r[:], h2[:], n[:], op=OP.subtract)
                    nc.vector.tensor_tensor(r[:], r[:], z[:], op=OP.mult)
                    out2 = wk.tile([B, DD], F32, tag=f"o2{name}", name=f"o2{name}")
                    nc.vector.tensor_add(out2[:], r[:], n[:])
                    oTp = ps_t64([128, 4, B], f"oT{name}")
                    for kc in range(4):
                        nc.tensor.transpose(oTp[:, kc, :],
                                            out2[:, kc * 128:(kc + 1) * 128],
                                            ident[:B, :B])
                    return out2, oTp

                sh2, shTp = gru(eT, 2, wih1_d, sT, s2, whh1_d, "1")
                shT = wk.tile([128, 4, B], F32, tag="shT", name="shT")
                nc.vector.tensor_copy(shT[:], shTp[:])
                new2, newTp = gru(ctxT, 4, wih2_d, shT, sh2, whh2_d, "2")
                nc.vector.tensor_copy(st2d[nxt][:], new2[:])
                nc.scalar.copy(stT[nxt][:], newTp[:])

                # ===== A. vocab projection + greedy argmax feedback =====
                svals = wk.tile([B, NT * 8], F32, tag="svals", name="svals")
                sidxf = wk.tile([B, NT * 8], F32, tag="sidxf", name="sidxf")
                for nt in range(NT):
                    pt = ps_fc()
                    for k in range(4):
                        nc.tensor.matmul(
                            pt[:], stT[nxt][:, k, :], fcwT[:, k, nt * VT:(nt + 1) * VT],
                            start=(k == 0), stop=(k == 3))
                    pr = wk.tile([B, VT], F32, tag="pr", bufs=2, name="pr")
                    if nt % 2 == 0:
                        nc.scalar.activation(pr[:], pt[:], AF.Copy,
                                             scale=maskpT[:, t:t + 1])
                    else:
                        nc.vector.tensor_scalar_mul(pr[:], pt[:],
                                                    maskpT[:, t:t + 1])
                    nc.sync.dma_start(preds_o[:, t, nt * VT:(nt + 1) * VT], pr[:])
                    if not last:
                        sl = slice(nt * 8, (nt + 1) * 8)
                        nc.vector.max(svals[:, sl], pr[:])
                        mi8 = wk.tile([B, 8], U32, tag="mi8", name="mi8")
                        nc.vector.max_index(mi8[:], svals[:, sl], pr[:])
                        nc.vector.tensor_copy(sidxf[:, sl], mi8[:])   # u32->f32
                        nc.vector.tensor_scalar_add(sidxf[:, sl], sidxf[:, sl],
                                                    float(nt * VT))
                if not last:
                    # within-core top-1 (value max, tie -> smallest index)
                    gmax = wk.tile([B, 1], F32, tag="gmax", name="gmax")
                    nc.vector.tensor_reduce(gmax[:], svals[:], axis=AX.X, op=OP.max)
                    meq = wk.tile([B, NT * 8], U8, tag="meq", name="meq")
                    nc.vector.tensor_scalar(meq[:], svals[:], gmax[:], None,
                                            op0=OP.is_equal)
                    seli = wk.tile([B, NT * 8], F32, tag="seli", name="seli")
                    nc.vector.select(seli[:], meq[:], sidxf[:], bigt[:])
                    lidx = wk.tile([B, 1], F32, tag="lidx", name="lidx")
                    nc.vector.tensor_reduce(lidx[:], seli[:], axis=AX.X, op=OP.min)
                    cand = wk.tile([B, 2], F32, tag="cand", name="cand")
                    nc.vector.tensor_copy(cand[:, 0:1], gmax[:])
                    nc.vector.tensor_tensor(cand[:, 1:2], lidx[:], vofff[:],
                                            op=OP.add)
                    nc.sync.dma_start(agk_in[:], cand[:])
                    nc.gpsimd.collective_compute(
                        "AllGather", OP.bypass, replica_groups=groups,
                        ins=[agk_in[:].opt()], outs=[agk_out[:].opt()])
                    call = wk.tile([B, NC, 2], F32, tag="call", name="call")
                    nc.sync.dma_start(
                        call[:], agk_out[:].rearrange("(c b) k -> b c k", c=NC))
                    g2 = wk.tile([B, 1], F32, tag="g2", name="g2")
                    nc.vector.tensor_reduce(g2[:], call[:, :, 0], axis=AX.X,
                                            op=OP.max)
                    m2 = wk.tile([B, NC], U8, tag="m2", name="m2")
                    nc.vector.tensor_scalar(m2[:], call[:, :, 0], g2[:], None,
                                            op0=OP.is_equal)
                    s2i = wk.tile([B, NC], F32, tag="s2i", name="s2i")
                    nc.vector.select(s2i[:], m2[:], call[:, :, 1], bigt[:, :NC])
                    wf = wk.tile([B, 1], F32, tag="wf", name="wf")
                    nc.vector.tensor_reduce(wf[:], s2i[:], axis=AX.X, op=OP.min)
                    wu = wk.tile([B, 1], U32, tag="wu", name="wu")
                    nc.vector.tensor_copy(wu[:], wf[:])
                    embn = wk.tile([B, EMB], F32, tag="embn", name="embn")
                    nc.gpsimd.indirect_dma_start(
                        out=embn[:], out_offset=None, in_=embed_d[:],
                        in_offset=bass.IndirectOffsetOnAxis(ap=wu[:, :1], axis=0))
                    etp = ps_t64([128, 2, B], "etp")
                    for kc in range(2):
                        nc.tensor.transpose(
                            etp[:, kc, :], embn[:, kc * 128:(kc + 1) * 128],
                            ident[:B, :B])
                    nc.scalar.copy(embT[nxt][:], etp[:])


    nc.finalize()
    return nc


_CACHE = {}


def _get_nc(Tmax):
    if Tmax not in _CACHE:
        _CACHE[Tmax] = build(Tmax)
    return _CACHE[Tmax]


def _chunkT(w, kchunks):
    """[out, in] weight -> lhsT/rhs SBUF layout [128, kchunks, out]."""
    o, i = w.shape
    assert i == kchunks * 128
    return np.ascontiguousarray(
        w.T.reshape(kchunks, 128, o).transpose(1, 0, 2)).astype(np.float32)


def kernel(encoder_out, encoded_captions, caption_lengths, embedding,
           gru1_w_ih, gru1_w_hh, gru1_b_ih, gru1_b_hh,
           gru2_w_ih, gru2_w_hh, gru2_b_ih, gru2_b_hh,
           enc_att_w, enc_att_b, dec_att_w, dec_att_b,
           conv_w, conv_b, conv_att_w, conv_att_b,
           full_att_w, full_att_b, s_w, s_b, fc_w, fc_b, _trace=False):
    encoder_out = np.asarray(encoder_out, np.float32)
    caption_lengths = np.asarray(caption_lengths)
    encoded_captions = np.asarray(encoded_captions)
    embedding = np.asarray(embedding, np.float32)
    f32 = lambda x: np.asarray(x, np.float32)
    gru1_w_ih, gru1_w_hh, gru2_w_ih, gru2_w_hh = map(
        f32, (gru1_w_ih, gru1_w_hh, gru2_w_ih, gru2_w_hh))
    enc_att_w, dec_att_w, conv_w, conv_att_w, full_att_w, s_w, fc_w = map(
        f32, (enc_att_w, dec_att_w, conv_w, conv_att_w, full_att_w, s_w, fc_w))
    for nm, bb in (("gru biases", (gru1_b_ih, gru1_b_hh, gru2_b_ih, gru2_b_hh)),
                   ("fc_b", (fc_b,))):
        for x in bb:
            assert not np.any(np.asarray(x)), f"nonzero {nm} not supported"

    sort_ind = np.argsort(-caption_lengths, kind="stable")
    lens = caption_lengths[sort_ind]
    enc = encoder_out[sort_ind]
    caps = encoded_captions[sort_ind]
    Tmax = int(lens.max())

    mean = enc.mean(axis=1, dtype=np.float32).astype(np.float32)
    s0 = (mean @ s_w.T + np.asarray(s_b, np.float32)).astype(np.float32)
    s0T = _chunkT(s0, 4)
    emb0 = np.ascontiguousarray(embedding[np.asarray(caps[:, 0], np.int64)])
    emb0T = _chunkT(emb0, 2)

    wih1 = _chunkT(gru1_w_ih, 2)
    whh1 = _chunkT(gru1_w_hh, 4)
    wih2 = _chunkT(gru2_w_ih, 4)
    whh2 = _chunkT(gru2_w_hh, 4)
    datwT = _chunkT(dec_att_w, 4)
    cawT = _chunkT(conv_att_w, 2)
    eawT = _chunkT(enc_att_w, 4)
    conv_w_pad = np.zeros((CD, 512), np.float32)
    conv_w_pad[:, :P] = conv_w
    cvwT = _chunkT(conv_w_pad, 4)
    eabT = np.ascontiguousarray(np.asarray(enc_att_b, np.float32).reshape(2, 128).T)
    ab0 = np.asarray(dec_att_b, np.float32)
    ab1 = (ab0 + np.asarray(conv_att_b, np.float32)
           + conv_att_w @ np.asarray(conv_b, np.float32)).astype(np.float32)
    ab0T = np.ascontiguousarray(ab0.reshape(2, 128).T)
    ab1T = np.ascontiguousarray(ab1.reshape(2, 128).T)
    wv = full_att_w[0].astype(np.float32)
    wsel = np.zeros((128, 2, BL, BL), np.float32)
    for ac in range(2):
        for b in range(BL):
            wsel[:, ac, b, b] = wv[ac * 128:(ac + 1) * 128]
    maskpT = (np.arange(Tmax)[None, :] < np.asarray(lens)[:, None]).astype(np.float32)
    maskpT = np.ascontiguousarray(maskpT)

    nc = _get_nc(Tmax)

    in_maps = []
    for c in range(NC):
        rows = slice(c * BL, (c + 1) * BL)
        enc_own = enc[rows]
        encP = np.zeros((128, 4, BL, ED), np.float32)
        ep = enc_own.transpose(1, 0, 2)   # [p, b, d]
        for pc in range(4):
            n = min(128, P - pc * 128)
            encP[:n, pc] = ep[pc * 128:pc * 128 + n]
        fcwT = _chunkT(fc_w[c * VL:(c + 1) * VL], 4)
        Sown = np.zeros((B, BL), np.float32)
        for j in range(BL):
            Sown[c * BL + j, j] = 1.0
        in_maps.append(dict(
            encP=encP, fcwT=fcwT, wih1=wih1, whh1=whh1, wih2=wih2, whh2=whh2,
            datwT=datwT, cvwT=cvwT, cawT=cawT, eawT=eawT, wsel=wsel,
            s0_2d=s0, s0T=s0T, emb0T=emb0T, eabT=eabT, ab0T=ab0T, ab1T=ab1T,
            Sown=Sown,
            vofff=np.full((B, 1), float(c * VL), np.float32),
            maskpT=maskpT,
            maskaT=np.ascontiguousarray(maskpT[rows]),
            embed=embedding,
        ))

    try:
        res = run_bass_kernel_spmd(nc, in_maps, core_ids=list(range(NC)),
                                   trace=_trace)
    except ModuleNotFoundError:
        # axon NTFF profile hook unavailable in this environment
        os.environ["BASS_NEVER_TRACE"] = "1"
        res = run_bass_kernel_spmd(nc, in_maps, core_ids=list(range(NC)),
                                   trace=False)
    predictions = np.concatenate(
        [res.results[c]["preds"] for c in range(NC)], axis=2)
    alphas = np.concatenate(
        [res.results[c]["alphas"] for c in range(NC)], axis=0)

    out = (predictions, caps, lens, alphas, sort_ind.astype(np.int32))
    if _trace:
        return out, res
    return out
